# revision 1
# baseline (speedup 1.0000x reference)
"""Causal self-attention (B=2, T=2048, C=768, H=12) on 8 TRN2 NeuronCores.

Sharding: core c = (b = c // 4, head-group hg = c % 4 of 3 heads).
Each core: QKV projection for its 3 heads (column-parallel), causal
attention, and a row-parallel slice of the output projection. The host
pre-transposes/casts operands to bf16 and sums the 4 partial outputs
per batch (row-parallel all-reduce done host-side) + bias.
"""

import os
import sys

import numpy as np
import ml_dtypes


def _ensure_paths():
    for p in ("/opt/trn_rl_repo", "/opt/pypackages"):
        if os.path.isdir(p) and p not in sys.path:
            sys.path.append(p)


_ensure_paths()

import concourse.bass as bass  # noqa: E402
import concourse.mybir as mybir  # noqa: E402
import concourse.tile as tile  # noqa: E402
from concourse import bacc  # noqa: E402
from concourse.bass_utils import run_bass_kernel_spmd  # noqa: E402
from concourse.masks import make_identity  # noqa: E402

BF16 = ml_dtypes.bfloat16

B, T, C, H, D = 2, 2048, 768, 12, 64
G = 3                # heads per core
FQK = 512            # q(192) | pad(64) | k(192) | pad(64) -> q/k same partition offsets
FV = G * D           # 192
NT = T // 128        # 16 token tiles
KS = C // 128        # 6 contraction subtiles

_cache: dict[bool, object] = {}
_last_in_maps = None


def _build(causal: bool):
    dt = mybir.dt
    nc = bacc.Bacc("TRN2", num_devices=8)

    xT_d = nc.dram_tensor("xT", [C, T], dt.bfloat16, kind="ExternalInput")
    wqkT_d = nc.dram_tensor("wqkT", [C, FQK], dt.bfloat16, kind="ExternalInput")
    wvT_d = nc.dram_tensor("wvT", [C, FV], dt.bfloat16, kind="ExternalInput")
    bqk_d = nc.dram_tensor("bqk", [128, 4], dt.float32, kind="ExternalInput")
    bv_d = nc.dram_tensor("bv", [128, FV], dt.float32, kind="ExternalInput")
    wpT_d = nc.dram_tensor("wpT", [256, C], dt.bfloat16, kind="ExternalInput")
    maskT_d = nc.dram_tensor("maskT", [128, 128], dt.bfloat16, kind="ExternalInput")
    out_d = nc.dram_tensor("out", [T, C], dt.float32, kind="ExternalOutput")

    Exp = mybir.ActivationFunctionType.Exp
    Log = mybir.ActivationFunctionType.Ln

    with tile.TileContext(nc) as tc:
        with tc.tile_pool(name="persist", bufs=1) as pp:
            xT_sb = pp.tile([128, KS, T], dt.bfloat16)
            wqkT_sb = pp.tile([128, KS, FQK], dt.bfloat16)
            wvT_sb = pp.tile([128, KS, FV], dt.bfloat16)
            wpT_sb = pp.tile([128, 2, C], dt.bfloat16)
            bqk_sb = pp.tile([128, 4], dt.float32)
            bv_sb = pp.tile([128, FV], dt.float32)
            maskT_sb = pp.tile([128, 128], dt.bfloat16)
            ones_sb = pp.tile([128, 64], dt.bfloat16)
            ident = pp.tile([128, 128], dt.bfloat16)
            y_sb = pp.tile([128, NT, FV], dt.bfloat16)
            qkT_sb = pp.tile([128, 4, T], dt.bfloat16)
            v_sb = pp.tile([128, NT, G, D + 1], dt.bfloat16)
            yT_sb = pp.tile([128, 2, T], dt.bfloat16)

            for s in range(KS):
                nc.sync.dma_start(
                    wqkT_sb[:, s, :],
                    wqkT_d.ap()[s * 128 : (s + 1) * 128, :],
                )
                nc.sync.dma_start(
                    xT_sb[:, s, :],
                    xT_d.ap()[s * 128 : (s + 1) * 128, :],
                )
            nc.sync.dma_start(
                wvT_sb[:], wvT_d.ap().rearrange("(s p) f -> p s f", p=128)
            )
            nc.sync.dma_start(
                wpT_sb[:], wpT_d.ap().rearrange("(s p) o -> p s o", p=128)
            )
            nc.sync.dma_start(bqk_sb[:], bqk_d.ap())
            nc.sync.dma_start(bv_sb[:], bv_d.ap())
            nc.sync.dma_start(maskT_sb[:], maskT_d.ap())
            nc.gpsimd.memset(ones_sb[:], 1.0)
            make_identity(nc, ident[:])

            # ---- Phase 1a: q/k projection -> qkT_sb [f, t] (bf16, +bias) ----
            with tc.tile_pool(name="ps_qk", bufs=3, space="PSUM") as qkps:
                for fi in range(4):
                    for tch in range(4):
                        ps = qkps.tile([128, 512], dt.float32)
                        for s in range(KS):
                            nc.tensor.matmul(
                                ps[:],
                                wqkT_sb[:, s, fi * 128 : (fi + 1) * 128],
                                xT_sb[:, s, tch * 512 : (tch + 1) * 512],
                                start=(s == 0),
                                stop=(s == KS - 1),
                            )
                        nc.vector.tensor_scalar_add(
                            qkT_sb[:, fi, tch * 512 : (tch + 1) * 512],
                            ps[:],
                            bqk_sb[:, fi : fi + 1],
                        )

            # ---- Phase 1b: v projection -> v_aug [t, g, d|1] (bf16, +bias) ----
            nc.gpsimd.memset(v_sb[:, :, :, D : D + 1], 1.0)
            with tc.tile_pool(name="ps_v", bufs=2, space="PSUM") as vps:
                for ti in range(NT):
                    ps = vps.tile([128, FV], dt.float32)
                    for s in range(KS):
                        nc.tensor.matmul(
                            ps[:],
                            xT_sb[:, s, ti * 128 : (ti + 1) * 128],
                            wvT_sb[:, s, :],
                            start=(s == 0),
                            stop=(s == KS - 1),
                        )
                    for h in range(G):
                        nc.vector.tensor_tensor(
                            v_sb[:, ti, h, 0:D],
                            ps[:, h * D : (h + 1) * D],
                            bv_sb[:, h * D : (h + 1) * D],
                            mybir.AluOpType.add,
                        )

            # ---- Phase 2: attention, 256-wide q chunks, per head ----
            # scores transposed [j, q]: each kT_j stationary load streams a
            # 256-wide q chunk (two q-tiles), halving score-matmul count; two
            # j-blocks share one score psum bank and two q-tiles share one
            # 130-col Y+denom bank (first writer start=True clears the bank,
            # the second overwrites its still-clear region). Transposes ride
            # the h1/h2 epilogues; out-projection + DMA follow h2 per q-tile.
            nc.gpsimd.memset(yT_sb[64:128, 1, :], 0.0)
            with (
                tc.tile_pool(name="ps_s", bufs=3, space="PSUM") as sps,
                tc.tile_pool(name="ps_y", bufs=2, space="PSUM") as yps,
                tc.tile_pool(name="ps_tr", bufs=1, space="PSUM") as trp,
                tc.tile_pool(name="ps_o", bufs=1, space="PSUM") as ops_,
                tc.tile_pool(name="pt", bufs=17) as ptp,
                tc.tile_pool(name="eps", bufs=4) as ep,
                tc.tile_pool(name="ob", bufs=3) as obp,
            ):
                for h in range(G):
                    qf = 64 * h
                    kf = 256 + 64 * h
                    qti, qoff = qf // 128, qf % 128
                    kti, koff = kf // 128, kf % 128
                    for qc in range(NT // 4):
                        q0 = qc * 512
                        jtop = 4 * qc + 4 if causal else NT
                        ypt = yps.tile([128, 260], dt.float32)
                        pts = []
                        for j in range(jtop):
                            w0 = max(j * 128, q0) if causal else q0
                            w = q0 + 512 - w0
                            sp = sps.tile([128, 512], dt.float32)
                            nc.tensor.matmul(
                                sp[:, 0:w],
                                qkT_sb[koff : koff + 64, kti, j * 128 : (j + 1) * 128],
                                qkT_sb[qoff : qoff + 64, qti, w0 : q0 + 512],
                                start=True,
                                stop=True,
                            )
                            pt = ptp.tile([128, 512], dt.bfloat16)
                            nc.scalar.activation(
                                pt[:, 0:w], sp[:, 0:w], Exp, scale=0.125
                            )
                            if causal and j * 128 >= q0:
                                nc.vector.tensor_mul(
                                    pt[:, 0:128], pt[:, 0:128], maskT_sb[:]
                                )
                            pts.append((pt, 0, w0))
                        for t_in in range(4):
                            qt = 4 * qc + t_in
                            c0 = t_in * 65
                            jmaxq = qt if causal else NT - 1
                            for j in range(jmaxq + 1):
                                pt, cstart, w0 = pts[j]
                                off = cstart + qt * 128 - w0
                                nc.tensor.matmul(
                                    ypt[:, c0 : c0 + 65],
                                    pt[:, off : off + 128],
                                    v_sb[:, j, h, :],
                                    start=(j == 0 and t_in == 0),
                                    stop=(j == jmaxq),
                                    skip_group_check=True,
                                )
                            rc = ep.tile([128, 1], dt.float32)
                            nc.vector.reciprocal(rc[:], ypt[:, c0 + D : c0 + D + 1])
                            nc.vector.tensor_scalar_mul(
                                y_sb[:, qt, h * D : (h + 1) * D],
                                ypt[:, c0 : c0 + D],
                                rc[:, 0:1],
                            )
                            if h == 1:
                                p1 = trp.tile([128, 128], dt.bfloat16, tag="tr")
                                nc.tensor.transpose(
                                    p1[:], y_sb[:, qt, 0:128], ident[:]
                                )
                                nc.vector.tensor_copy(
                                    yT_sb[:, 0, qt * 128 : (qt + 1) * 128], p1[:]
                                )
                            elif h == 2:
                                p2 = trp.tile([128, 128], dt.bfloat16, tag="tr")
                                nc.tensor.transpose(
                                    p2[0:64, :], y_sb[:, qt, 128:192], ident[:]
                                )
                                nc.vector.tensor_copy(
                                    yT_sb[0:64, 1, qt * 128 : (qt + 1) * 128],
                                    p2[0:64, :],
                                )
                                po1 = ops_.tile([128, 384], dt.float32, tag="po1")
                                po2 = ops_.tile([128, 384], dt.float32, tag="po2")
                                for s in range(2):
                                    nc.tensor.matmul(
                                        po1[:],
                                        yT_sb[:, s, qt * 128 : (qt + 1) * 128],
                                        wpT_sb[:, s, 0:384],
                                        start=(s == 0),
                                        stop=(s == 1),
                                    )
                                    nc.tensor.matmul(
                                        po2[:],
                                        yT_sb[:, s, qt * 128 : (qt + 1) * 128],
                                        wpT_sb[:, s, 384:768],
                                        start=(s == 0),
                                        stop=(s == 1),
                                    )
                                ob = obp.tile([128, C], dt.float32)
                                nc.vector.tensor_copy(ob[:, 0:384], po1[:])
                                nc.vector.tensor_copy(ob[:, 384:768], po2[:])
                                nc.sync.dma_start(
                                    out_d.ap()[qt * 128 : (qt + 1) * 128, :], ob[:]
                                )

    nc.compile()
    return nc


def _prep_in_maps(x, Wqkv, bqkv, Wproj):
    in_maps = []
    for c in range(8):
        b, hg = c // 4, c % 4
        r0 = 192 * hg
        xT = np.ascontiguousarray(x[b].T).astype(BF16)
        wqk = np.zeros((512, 768), dtype=np.float32)
        wqk[0:192] = Wqkv[r0 : r0 + 192]
        wqk[256:448] = Wqkv[768 + r0 : 768 + r0 + 192]
        wqkT = np.ascontiguousarray(wqk.T).astype(BF16)
        wvT = np.ascontiguousarray(Wqkv[1536 + r0 : 1536 + r0 + 192].T).astype(BF16)
        bqk_vec = np.zeros(512, dtype=np.float32)
        bqk_vec[0:192] = bqkv[r0 : r0 + 192]
        bqk_vec[256:448] = bqkv[768 + r0 : 768 + r0 + 192]
        bqk = np.ascontiguousarray(bqk_vec.reshape(4, 128).T.astype(np.float32))
        bv = np.tile(
            bqkv[1536 + r0 : 1536 + r0 + 192].astype(np.float32)[None, :], (128, 1)
        )
        wp = np.zeros((256, 768), dtype=BF16)
        wp[0:192] = Wproj[:, r0 : r0 + 192].T.astype(BF16)
        maskT = np.triu(np.ones((128, 128), dtype=np.float32)).astype(BF16)
        in_maps.append(
            {
                "xT": xT,
                "wqkT": np.ascontiguousarray(wqkT),
                "wvT": wvT,
                "bqk": np.ascontiguousarray(bqk),
                "bv": bv,
                "wpT": wp,
                "maskT": maskT,
            }
        )
    return in_maps


def kernel(x, Wqkv, bqkv, Wproj, bproj, is_causal):
    global _last_in_maps
    x = np.asarray(x, dtype=np.float32)
    Wqkv = np.asarray(Wqkv, dtype=np.float32)
    bqkv = np.asarray(bqkv, dtype=np.float32)
    Wproj = np.asarray(Wproj, dtype=np.float32)
    bproj = np.asarray(bproj, dtype=np.float32)
    causal = bool(int(np.asarray(is_causal)))

    if causal not in _cache:
        _cache[causal] = _build(causal)
    nc = _cache[causal]

    in_maps = _prep_in_maps(x, Wqkv, bqkv, Wproj)
    _last_in_maps = in_maps
    res = run_bass_kernel_spmd(nc, in_maps, core_ids=list(range(8)))

    out = np.empty((B, T, C), dtype=np.float32)
    for b in range(B):
        acc = res.results[4 * b]["out"].copy()
        for k in range(1, 4):
            acc += res.results[4 * b + k]["out"]
        out[b] = acc + bproj[None, :]
    return out

